# revision 1
# baseline (speedup 1.0000x reference)
"""CapsuleLayer dynamic-routing kernel for 8 TRN2 NeuronCores.

Problem: inputs [256,1152,8] f32, W [1152,10,8,16] f32, bias [1,1152,10,1] f32.
  u_hat = einsum('bid,icdv->bicv', inputs, W)
  3 rounds of routing (softmax over c, weighted sum over i, squash over v).
Output: [256, 10, 16] f32.

Sharding: 2-way batch x 4-way input-capsule (i) grid over 8 cores.
Core k: batch half k//4 (128 rows), i-quarter k%4 (288 i's).
Per-round partial sums over i are combined with an AllReduce over each
group of 4 cores ([0..3] and [4..7]). Output halves read from cores 0, 4.

v2 design: the weighted-sum stage s = sum_i c*u_hat runs on the PE
instead of DVE/Pool chains:
  - coupling weights cw[b,(c,j)] are transposed by PE identity matmuls
    into cwT[(c,j), b] tiles,
  - xcT[(c,d,j), b] = cwT * xT elementwise (DVE, 2x bf16),
  - s^T[(c,v), b] accumulates in PSUM via dense [96j,16v] W2 matmuls,
  - tiny [16,128] PE transposes bring s back to batch-partitioned form.
Logits l = sum_v u_hat*v remain mul+halving-tree chunks on DVE with some
chunks offloaded to Pool. exp on Act scatters (j,c)->(c,j) so softmax
tail ops stay in DVE 2x mode.
"""

import sys

if "/opt/trn_rl_repo" not in sys.path:
    sys.path.insert(0, "/opt/trn_rl_repo")

import numpy as np
import ml_dtypes

import concourse.bass as bass
from concourse import bacc, mybir, tile
from concourse.bass_utils import run_bass_kernel_spmd

F32 = mybir.dt.float32
BF16 = mybir.dt.bfloat16
AX = mybir.AxisListType
ALU = mybir.AluOpType
ACTF = mybir.ActivationFunctionType

B, I, D, C, V = 256, 1152, 8, 10, 16
CV = C * V                     # 160
NB = 128                       # batch rows per core
IQ = 288                       # i's per core ("j" below)
NG = IQ // 4                   # 72 groups of 4 i's (K=32 block-diag matmuls)
NT = NG // 3                   # 24 tiles of 96 partition-rows
EPS = 1e-7

# logits chunk plan: (j-start, width, on_pool); Pool keeps narrow chunks so
# its scratch stays small, DVE uses wide chunks to amortize per-op overheads
# logits chunks: 18 of width 16; the first four run on Pool
CHUNK_PLAN = [(16 * k, 16, k < 4) for k in range(18)]
Z_PLAN = [(48 * b, 48) for b in range(6)]
JT = 3                         # 96-row j-tiles per (c) or (d)

REPLICA_GROUPS = [[0, 1, 2, 3], [4, 5, 6, 7]]

# logits chunks offloaded to Pool (gpsimd), per round

# xc muls offloaded to Pool in the ws phase (empty: a slow Pool mul stalls
# the in-order PE queue and resets its pstate ramp)
POOL_XC = set()


def _ap(ap, dims):
    """Build an AP with explicit [step, count] free dims (partition dim kept)."""
    return bass.AP(ap.tensor, ap.offset, [list(ap.ap[0])] + [list(d) for d in dims])


def _squash(nc, pool, s_in, v_out):
    """v = (|s|^2/(1+|s|^2)) * s / sqrt(|s|^2 + EPS), norms over v (16).

    s_in: [128, 160] f32-ish SBUF AP. Writes v_out (bf16 for routing rounds,
    f32 for the final output round)."""
    sq = pool.tile([128, CV], F32, tag="sq")
    n2 = pool.tile([128, C], F32, tag="n2")
    n2e = pool.tile([128, C], F32, tag="n2e")
    qs = pool.tile([128, C], F32, tag="qs")
    mm = pool.tile([128, C], F32, tag="mm")
    rm = pool.tile([128, C], F32, tag="rm")
    fc = pool.tile([128, C], F32, tag="fc")
    nc.vector.tensor_mul(sq[:], s_in, s_in)
    nc.vector.tensor_reduce(
        n2[:], sq[:].rearrange("p (c v) -> p c v", v=V), axis=AX.X, op=ALU.add
    )
    # f = n2 / ((1+n2) * sqrt(n2+eps))
    nc.vector.tensor_scalar_add(n2e[:], n2[:], EPS)
    nc.scalar.activation(qs[:], n2e[:], ACTF.Sqrt)
    nc.vector.scalar_tensor_tensor(
        mm[:], n2[:], 1.0, qs[:], op0=ALU.add, op1=ALU.mult
    )
    nc.vector.reciprocal(rm[:], mm[:])
    nc.vector.tensor_mul(fc[:], n2[:], rm[:])
    # v = s * f (broadcast f over v)
    f_b = _ap(fc[:], [[1, C], [0, V]])
    s3 = s_in.rearrange("p (c v) -> p c v", v=V)
    nc.vector.tensor_mul(v_out[:].rearrange("p (c v) -> p c v", v=V), s3, f_b)


def _emit(nc, tc, use_bias, cc_stub=False):
    xt2_d = nc.declare_dram_parameter("xt2", [96, NT * 128], BF16, isOutput=False)
    wbd_d = nc.declare_dram_parameter("wbd", [96, NT * 640], BF16, isOutput=False)
    w2d_d = nc.declare_dram_parameter("w2d", [96, NT * CV], BF16, isOutput=False)
    xt_d = nc.declare_dram_parameter("xt", [96, 24 * 128], BF16, isOutput=False)
    w2_d = nc.declare_dram_parameter("w2", [96, 240 * 16], BF16, isOutput=False)
    id_d = nc.declare_dram_parameter("ident", [128, 128], BF16, isOutput=False)
    id32_d = nc.declare_dram_parameter("ident32", [128, 128], F32, isOutput=False)
    if use_bias:
        bias_d = nc.declare_dram_parameter("biasr", [128, IQ * C], BF16, isOutput=False)
    out_d = nc.declare_dram_parameter("out", [128, CV], F32, isOutput=True)

    WBC = 3                    # NT-tiles per streamed wbd chunk
    NWB = NT // WBC            # 8 chunks

    with (
        tc.tile_pool(name="const", bufs=1) as cp,
        tc.tile_pool(name="wbd_ring", bufs=2) as wp,
        tc.tile_pool(name="small", bufs=1) as sp,
        tc.tile_pool(name="ring", bufs=2) as rp,
        tc.tile_pool(name="gscr", bufs=1) as gp_scr,
        tc.tile_pool(name="xcr", bufs=3) as xp,
        tc.tile_pool(name="cwt", bufs=1) as cwp,
        tc.tile_pool(name="ps0", bufs=1, space="PSUM") as ps0p,
        tc.tile_pool(name="psg", bufs=3, space="PSUM") as psgp,
        tc.tile_pool(name="psT", bufs=1, space="PSUM") as psTp,
        tc.tile_pool(name="dram", bufs=1, space="DRAM") as dp,
    ):
        xt2 = cp.tile([96, NT * 128], BF16, tag="xt2")
        w2d = cp.tile([96, NT * CV], BF16, tag="w2d")
        xt = cp.tile([96, 24 * 128], BF16, tag="xt")
        w2 = cp.tile([96, 240 * 16], BF16, tag="w2")
        ident = cp.tile([128, 128], BF16, tag="ident")
        ident32 = cp.tile([128, 128], F32, tag="ident32")
        uhat = cp.tile([128, IQ * CV], BF16, tag="uhat")

        xsl = slice(0, 6 * 128)
        nc.sync.dma_start(xt2[:, xsl], xt2_d[:, xsl])
        wbd_tiles = []
        for wch in range(NWB):
            wt = wp.tile([96, WBC * 640], BF16, tag="wbd")
            if wch < 2:
                nc.sync.dma_start(
                    wt[:], wbd_d[:, wch * WBC * 640 : (wch + 1) * WBC * 640]
                )
            wbd_tiles.append(wt)
        for xch in range(1, 4):
            xsl = slice(xch * 6 * 128, (xch + 1) * 6 * 128)
            nc.sync.dma_start(xt2[:, xsl], xt2_d[:, xsl])
        nc.sync.dma_start(w2d[:], w2d_d[:])
        for wch in range(2, NWB):
            nc.sync.dma_start(
                wbd_tiles[wch][:],
                wbd_d[:, wch * WBC * 640 : (wch + 1) * WBC * 640],
            )
        nc.sync.dma_start(xt[:], xt_d[:])
        nc.sync.dma_start(w2[:], w2_d[:])
        nc.sync.dma_start(ident[:], id_d[:])
        nc.sync.dma_start(ident32[:], id32_d[:])
        if use_bias:
            biasr = cp.tile([128, IQ * C], BF16, tag="biasr")
            nc.sync.dma_start(biasr[:], bias_d[:])

        # persistent small tiles
        warm = sp.tile([128, 1], F32, tag="warm")
        nc.vector.memset(warm[:], 1.0)

        warm2 = sp.tile([128, 1], F32, tag="warm2")

        def prewarm(func, anchor=None):
            # dummy op so the ACT table set loads off the critical path;
            # anchor ties it to freshly-written data so the tile scheduler
            # cannot hoist it ahead of the ops it must follow
            if anchor is None:
                nc.scalar.activation(warm2[:], warm[:], func)
            else:
                # scale=0 keeps the data dependency but feeds func(0)
                nc.scalar.activation(warm2[:], anchor, func, scale=0.0)

        prewarm(ACTF.Sqrt)
        v_f = sp.tile([128, CV], F32, tag="v_f")
        v_b = sp.tile([128, CV], BF16, tag="v_b")
        s_part = sp.tile([128, CV], F32, tag="s_part")
        sTe = sp.tile([128, 3 * 128], BF16, tag="sTe")
        s_part32 = sp.tile([128, CV], F32, tag="s_part32")
        s_tot = sp.tile([128, CV], F32, tag="s_tot")
        ta = sp.tile([128, IQ * C], BF16, tag="ta")   # raw logits (j,c), rotates
        tb = sp.tile([128, IQ * C], BF16, tag="tb")
        et = sp.tile([128, IQ * C], BF16, tag="et")   # exp(logits), (c,j) order
        cw = sp.tile([128, IQ * C], BF16, tag="cw")   # softmax couplings (c,j)
        zsum = sp.tile([128, IQ], F32, tag="zsum")
        rz = sp.tile([128, IQ], BF16, tag="rz")

        def all_reduce(rnd, src, dst, dt, n):
            ccin = dp.tile([128, n], dt, tag=f"ccin{rnd}")
            ccout = dp.tile([128, n], dt, tag=f"ccout{rnd}")
            nc.scalar.dma_start(ccin[:], src[:, :n])
            if cc_stub:
                nc.scalar.dma_start(ccout[:], ccin[:])
            else:
                nc.gpsimd.collective_compute(
                    "AllReduce",
                    ALU.add,
                    replica_groups=REPLICA_GROUPS,
                    ins=[ccin.opt()],
                    outs=[ccout.opt()],
                )
            nc.scalar.dma_start(dst[:, :n], ccout[:])

        # ---- u_hat generation: block-diag matmuls, 4 i's per PSUM chunk
        def gen_group(g):
            ps = psgp.tile([128, 1024], F32, tag="psg")
            t, s = divmod(g, 3)
            wt = wbd_tiles[t // WBC]
            wof = (t % WBC) * 640
            for half in range(2):     # j0/j1 cols then j2/j3 cols
                nc.tensor.matmul(
                    ps[:, half * 512 :][:, :320],
                    xt2[s * 32 : (s + 1) * 32, t * 128 : (t + 1) * 128],
                    wt[s * 32 : (s + 1) * 32, wof + half * 320 :][:, :320],
                    start=True,
                    stop=True,
                )
            src = ps[:].rearrange("p (b x) -> p b x", b=2)[:, :, :320]
            dst = uhat[:, g * 640 : (g + 1) * 640].rearrange(
                "p (b x) -> p b x", b=2
            )
            if g < 32 and g % 2 == 0:
                nc.vector.tensor_copy(dst, src)     # startup: DVE has slack
            else:
                nc.scalar.copy(dst, src)

        # early gen groups fill the pre-v0 idle window on PE/DVE
        for g in range(8):
            gen_group(g)

        # ---- round 0: s0 = sum_i softmax_c(bias)[i,c] * u_hat; the softmax
        # weights are folded into w2d on the host (uniform 1/C for zero bias)
        ps0 = ps0p.tile([128, CV], F32, tag="pscom")
        for t in range(NT):
            nc.tensor.matmul(
                ps0[:],
                xt2[:, t * 128 : (t + 1) * 128],
                w2d[:, t * CV : (t + 1) * CV],
                start=(t == 0),
                stop=(t == NT - 1),
            )
        nc.vector.tensor_copy(s_part[:], ps0[:])
        all_reduce(0, s_part, s_tot, F32, CV)
        # bulk u_hat generation: all groups issued up front; DVE takes the
        # even evictions below g=40, Act the rest, streaming through the AR0
        # window and the first half of round 1
        for g in range(8, 31):
            gen_group(g)
        _squash(nc, sp, s_tot[:], v_b)
        for g in range(31, NG):
            gen_group(g)
        prewarm(ACTF.Exp, v_b[:, 0:1])

        # ---- routing rounds 1, 2
        lg1 = None
        for rnd in (1, 2):
            raw = ta if rnd == 1 else tb

            # phase 1: logits chunks l[b,(j,c)] = sum_v u_hat*v  (DVE/Pool)
            for j0, w, on_pool in CHUNK_PLAN:
                eng = nc.gpsimd if on_pool else nc.vector
                ks = slice(j0 * C, (j0 + w) * C)
                uh = uhat[:, j0 * CV : (j0 + w) * CV]
                if on_pool:
                    tmp = gp_scr.tile([128, 16 * CV], BF16, tag="ringg")
                    tre = gp_scr.tile([128, 1600], BF16, tag="treeg")
                else:
                    tmp = rp.tile([128, 16 * CV], BF16, tag="ring")
                    tre = rp.tile([128, 1600], BF16, tag="tree")
                h8 = w * 80            # elems in the v=8 tree level
                vb3 = _ap(v_b[:], [[0, w], [16, C], [1, V]])
                eng.tensor_mul(
                    tmp[:, : w * CV].rearrange("p (i c v) -> p i c v", c=C, v=V),
                    uh.rearrange("p (i c v) -> p i c v", c=C, v=V),
                    vb3,
                )
                t16 = tmp[:, : w * CV].rearrange("p (x v) -> p x v", v=16)
                t8 = tre[:, 0:h8].rearrange("p (x v) -> p x v", v=8)
                t4 = tmp[:, 0 : h8 // 2].rearrange("p (x v) -> p x v", v=4)
                t2 = tre[:, h8 : h8 + h8 // 4].rearrange("p (x v) -> p x v", v=2)
                eng.tensor_add(t8, t16[:, :, 0:8], t16[:, :, 8:16])
                eng.tensor_add(t4, t8[:, :, 0:4], t8[:, :, 4:8])
                eng.tensor_add(t2, t4[:, :, 0:2], t4[:, :, 2:4])
                eng.tensor_add(
                    raw[:, ks],
                    t2[:, :, 0:1].rearrange("p x v -> p (x v)"),
                    t2[:, :, 1:2].rearrange("p x v -> p (x v)"),
                )
                if rnd == 1 and use_bias:
                    eng.tensor_add(raw[:, ks], raw[:, ks], biasr[:, ks])
                if rnd == 2:
                    eng.tensor_add(raw[:, ks], raw[:, ks], lg1[:, ks])
                # exp scatters (j,c) -> (c,j), batched per 3 chunks to cut
                # Act per-op overhead (Act co-paces round 1 with evictions)
                if j0 % 48 == 32:
                    jb = j0 - 32
                    raw_k = _ap(raw[:, jb * C :], [[C, 48], [1, C]])
                    et_k = _ap(et[:, jb:], [[1, 48], [IQ, C]])
                    nc.scalar.activation(et_k, raw_k, ACTF.Exp)
            # remaining softmax partition sums, deferred past the chunk muls
            zrem = Z_PLAN
            for jz, wz in zrem:
                kz = slice(jz, jz + wz)
                ein = _ap(et[:, jz:], [[1, wz], [IQ, C]])
                nc.vector.tensor_reduce(zsum[:, kz], ein, axis=AX.X, op=ALU.add)
                with nc.allow_low_precision(reason="1/z to bf16: couplings tolerate 0.4% scale noise"):
                    nc.vector.reciprocal(rz[:, kz], zsum[:, kz])

            # softmax tail: cw[b,(c,j)] = et * (1/z) broadcast over c (2x),
            # in 2-c slices so the PE transposes start on the first slice
            for cf in range(5):
                co = cf * 2 * IQ
                rz_b = _ap(rz[:], [[0, 2], [1, IQ]])
                nc.vector.tensor_mul(
                    cw[:, co : co + 2 * IQ].rearrange("p (c j) -> p c j", c=2),
                    et[:, co : co + 2 * IQ].rearrange("p (c j) -> p c j", c=2),
                    rz_b,
                )

            # transpose cw -> cwT[(c,j), b] in 5 psum fills of 6 tiles (2 c's)
            cwT_tiles = []
            for f in range(5):
                pst = psgp.tile([96, 6 * 128], F32, tag="psg")
                cwt = cwp.tile([96, 6 * 128], BF16, tag=f"cwt{f}")
                for t6 in range(6):
                    tix = f * 6 + t6      # global tile (c, jt) index = c*3+jt
                    nc.tensor.matmul(
                        pst[:, t6 * 128 : (t6 + 1) * 128],
                        cw[:, tix * 96 : (tix + 1) * 96],
                        ident[:],
                        start=True,
                        stop=True,
                        tile_position=(0, 0),
                    )
                nc.scalar.copy(cwt[:], pst[:])
                cwT_tiles.append(cwt)

            # ws stage: per c, xcT = cwT*xT (DVE), 24 PE matmuls into psum
            sT = psTp.tile([128, 3 * 128], F32, tag="sT")
            nc.vector.memset(sT[:], 0.0)
            for c in range(C):
                cwt = cwT_tiles[c // 2]
                cof = (c % 2) * JT * 128
                xcq = xp.tile([96, 24 * 128], BF16, tag="xcq")
                in0 = _ap(cwt[:, cof:], [[0, D], [128, JT], [1, 128]])
                in1 = _ap(xt[:], [[JT * 128, D], [128, JT], [1, 128]])
                out = _ap(xcq[:], [[JT * 128, D], [128, JT], [1, 128]])
                xeng = nc.gpsimd if c in POOL_XC else nc.vector
                xeng.tensor_mul(out, in0, in1)
                pb = 32 * (c % 4)
                cb = 128 * (c // 4)
                for dd in range(D):
                    for jt in range(JT):
                        tk = dd * JT + jt
                        nc.tensor.matmul(
                            sT[pb : pb + 16, cb : cb + 128],
                            w2[:, (c * 24 + tk) * 16 : (c * 24 + tk + 1) * 16],
                            xcq[:, tk * 128 : (tk + 1) * 128],
                            start=(tk == 0),
                            stop=(tk == 23),
                            tile_position=(0, pb),
                        )

            # transpose sTe back to batch-partitioned form: 3 full-width
            # PE transposes (lhsT at base 0; offset-sliced lhsT reads crash
            # the HW path), then strided gathers into s_part order
            psr = ps0p.tile([128, 3 * 128], F32, tag="pscom")
            nc.scalar.copy(sTe[:], sT[:])
            for jg in range(3):
                nc.tensor.matmul(
                    psr[:, jg * 128 : (jg + 1) * 128],
                    sTe[:, jg * 128 : (jg + 1) * 128],
                    ident[:],
                    start=True,
                    stop=True,
                    tile_position=(0, 0),
                )
            prewarm(ACTF.Sqrt, sTe[:, 0:1])   # table loads during the AR
            spart = s_part if rnd == 1 else s_part32
            for jg in range(3):
                ncv = 4 if jg < 2 else 2
                dstg = _ap(spart[:, jg * 64 :], [[16, ncv], [1, 16]])
                srcg = _ap(psr[:, jg * 128 :], [[32, ncv], [1, 16]])
                nc.vector.tensor_copy(dstg, srcg)
            all_reduce(rnd, spart, s_tot, F32, CV)
            if rnd == 1:
                _squash(nc, sp, s_tot[:], v_b)
                lg1 = raw
                prewarm(ACTF.Exp, v_b[:, 0:1])
            else:
                _squash(nc, sp, s_tot[:], v_f)

        nc.sync.dma_start(out_d[:], v_f[:])


_PROGRAMS = {}


def _get_program(use_bias=False, cc_stub=False):
    key = (use_bias, cc_stub)
    if key not in _PROGRAMS:
        nc = bacc.Bacc(
            "TRN2", target_bir_lowering=False, debug=False, num_devices=8
        )
        with tile.TileContext(nc) as tc:
            _emit(nc, tc, use_bias, cc_stub)
        nc.compile()
        _PROGRAMS[key] = nc
    return _PROGRAMS[key]


def make_in_maps(inputs, W, bias):
    assert tuple(np.shape(inputs)) == (B, I, D), np.shape(inputs)
    assert tuple(np.shape(W)) == (I, C, D, V), np.shape(W)
    assert tuple(np.shape(bias)) == (1, I, C, 1), np.shape(bias)
    use_bias = bool(np.any(np.asarray(bias)))
    in_maps = []
    for k in range(8):
        bh, iq = k // 4, k % 4
        xs = np.asarray(inputs[bh * NB : (bh + 1) * NB, iq * IQ : (iq + 1) * IQ, :])
        ws = np.asarray(W[iq * IQ : (iq + 1) * IQ])  # [288, 10, 8, 16]

        xT = xs.reshape(NB, IQ * D).T  # [2304, 128] rows (j,d)
        xt2 = xT.reshape(NT, 96, NB).transpose(1, 0, 2).reshape(96, NT * NB)

        Wt = ws.transpose(0, 2, 1, 3)  # [288, 8, 10, 16] (j, d, c, v)
        bs = np.asarray(bias[0, iq * IQ : (iq + 1) * IQ, :, 0], dtype=np.float64)
        eb = np.exp(bs - bs.max(axis=1, keepdims=True))
        cb = (eb / eb.sum(axis=1, keepdims=True)).astype(np.float32)  # [288, 10]
        Wt_s = Wt * cb[:, None, :, None]  # fold round-0 softmax into s0 weights
        w2dense = Wt_s.reshape(IQ * D, CV)  # [(j,d), (c,v)]
        w2d = w2dense.reshape(NT, 96, CV).transpose(1, 0, 2).reshape(96, NT * CV)

        bd = np.zeros((NG, 32, 640), dtype=np.float32)
        Wg = Wt.reshape(NG, 4, D, CV)
        for j in range(4):
            bd[:, j * D : (j + 1) * D, j * CV : (j + 1) * CV] = Wg[:, j]
        wbd = bd.reshape(NT, 96, 640).transpose(1, 0, 2).reshape(96, NT * 640)

        # v2: xT in (d, j) row order, 24 tiles of 96 rows -> [96, 24*128]
        A = xs.transpose(2, 1, 0).reshape(D * IQ, NB)  # row = d*288+j
        xt = A.reshape(24, 96, NB).transpose(1, 0, 2).reshape(96, 24 * 128)

        # v2: ws weights, dense [96, 16] tiles indexed (c*24 + d*3 + jt)
        Wc = ws.transpose(1, 2, 0, 3)  # [c, d, j, v]
        w2t = Wc.reshape(C, D, JT, 96, V).transpose(3, 0, 1, 2, 4)
        w2 = w2t.reshape(96, C * D * JT * V)

        ident = np.eye(128, dtype=np.float32)

        m = {
            "xt2": np.ascontiguousarray(xt2).astype(ml_dtypes.bfloat16),
            "wbd": np.ascontiguousarray(wbd).astype(ml_dtypes.bfloat16),
            "w2d": np.ascontiguousarray(w2d).astype(ml_dtypes.bfloat16),
            "xt": np.ascontiguousarray(xt).astype(ml_dtypes.bfloat16),
            "w2": np.ascontiguousarray(w2).astype(ml_dtypes.bfloat16),
            "ident": ident.astype(ml_dtypes.bfloat16),
            "ident32": ident,
        }
        if use_bias:
            bs = np.asarray(bias[0, iq * IQ : (iq + 1) * IQ, :, 0])
            biasr = np.broadcast_to(bs.reshape(1, IQ * C), (128, IQ * C))
            m["biasr"] = np.ascontiguousarray(biasr).astype(ml_dtypes.bfloat16)
        in_maps.append(m)
    return use_bias, in_maps


def run(inputs, W, bias, **kw):
    use_bias, in_maps = make_in_maps(inputs, W, bias)
    nc = _get_program(use_bias)
    res = run_bass_kernel_spmd(nc, in_maps, core_ids=list(range(8)), **kw)
    outs = res.results
    o0 = np.asarray(outs[0]["out"], dtype=np.float32).reshape(NB, C, V)
    o1 = np.asarray(outs[4]["out"], dtype=np.float32).reshape(NB, C, V)
    return np.concatenate([o0, o1], axis=0), res


def kernel(inputs, W, bias):
    out, _ = run(inputs, W, bias)
    return out



# revision 39
# speedup vs baseline: 1.1088x; 1.1088x over previous
"""CapsuleLayer dynamic-routing kernel for 8 TRN2 NeuronCores (v3).

Problem: inputs [256,1152,8] f32, W [1152,10,8,16] f32, bias [1,1152,10,1] f32.
  u_hat = einsum('bid,icdv->bicv', inputs, W)
  3 rounds of routing (softmax over c, weighted sum over i, squash over v).
Output: [256, 10, 16] f32.

Sharding: 2-way batch x 4-way input-capsule (i) grid over 8 cores.
Core k: batch half k//4 (128 rows), i-quarter k%4 (288 i's = "j").

v3 design: u_hat is never materialized. Logits use the identity
  l[b,j,c] = sum_d x[b,j,d] * g[b,j,c,d],  g = sum_v W[j,c,d,v] v[b,c,v]
where g comes from PE matmuls with shared weights (k=v=16) against the
transposed routing vector vT. This kills the u_hat generation matmuls,
the PSUM evictions of u_hat, and halves the DVE logits work (the
reduction runs over d=8 instead of v=16).

Per round: g psum chunks [128,1024] -> Act evicts to bf16 -> DVE muls
x (.) g -> d-halving-tree -> l[b,(c,j)] (c-major, so exp/softmax slices
are contiguous). Softmax: exp on Act, z-tree + approx-reciprocal + cw
on DVE. cw is PE-transposed into bf16 PSUM and consumed there by the
ws-stage muls (2x_1p mode); dense [96k,16] matmuls accumulate sT[(c,v),b].
Rounds pipeline per c-group {0-3, 4-7, 8-9}: each group's sT transposes
back, AllReduces over the 4-core replica group, squashes, and re-enters
the next round's g matmuls while later groups still run their ws stage.

Squash avoids Sqrt (keeps a single Act table set: copy/exp/ln/square):
  1/sqrt(n2+eps) = exp(-0.5*ln(n2+eps)).
"""

import sys

if "/opt/trn_rl_repo" not in sys.path:
    sys.path.insert(0, "/opt/trn_rl_repo")

import numpy as np
import ml_dtypes

import concourse.bass as bass
from concourse import bacc, mybir, tile
from concourse.bass_utils import run_bass_kernel_spmd

F32 = mybir.dt.float32
BF16 = mybir.dt.bfloat16
AX = mybir.AxisListType
ALU = mybir.AluOpType
ACTF = mybir.ActivationFunctionType

B, I, D, C, V = 256, 1152, 8, 10, 16
CV = C * V                     # 160
NB = 128                       # batch rows per core
IQ = 288                       # i's per core ("j" below)
JT = 3                         # 96-row j-tiles per (c) or (d)
KW = IQ * D                    # 2304 contraction cols per c
GTOT = C * KW                  # 23040 global (c,j,d) columns
CH = 1024                      # g psum chunk width (2 banks)
NCH = (GTOT + CH - 1) // CH    # 23 chunks
EPS = 1e-7

REPLICA_GROUPS = [[0, 1, 2, 3], [4, 5, 6, 7]]

# c-groups: sT/vT psum packing wants 32-aligned partition strips (4 c's
# per 128-partition tile); group A/B boundaries also land on whole g-chunks
# (4*2304 = 9*1024).
GROUPS = [(0, 4), (4, 4), (8, 2)]   # (first c, n c's)


def _ap(ap, dims):
    """Build an AP with explicit [step, count] free dims (partition dim kept)."""
    return bass.AP(ap.tensor, ap.offset, [list(ap.ap[0])] + [list(d) for d in dims])


def _emit(nc, tc, use_bias, cc_stub=False):
    xt_d = nc.declare_dram_parameter("xt", [96, 24 * 128], BF16, isOutput=False)
    w2d0_d = nc.declare_dram_parameter("w2d0", [96, 24 * CV], BF16, isOutput=False)
    wg_d = nc.declare_dram_parameter("wg", [16, GTOT], BF16, isOutput=False)
    xb_d = nc.declare_dram_parameter("xb", [128, KW], BF16, isOutput=False)
    w2_d = nc.declare_dram_parameter("w2", [96, 240 * 16], BF16, isOutput=False)
    id_d = nc.declare_dram_parameter("ident", [128, 128], BF16, isOutput=False)
    id32_d = nc.declare_dram_parameter("ident32", [128, 128], F32, isOutput=False)
    if use_bias:
        bias_d = nc.declare_dram_parameter("biasr", [128, IQ * C], BF16, isOutput=False)
    out_d = nc.declare_dram_parameter("out", [128, CV], F32, isOutput=True)

    with (
        tc.tile_pool(name="const", bufs=1) as cp,
        tc.tile_pool(name="small", bufs=1) as sp,
        tc.tile_pool(name="gbr", bufs=3) as gbp,
        tc.tile_pool(name="xcr", bufs=3) as xp,
        tc.tile_pool(name="tmpr", bufs=2) as tp,
        tc.tile_pool(name="tl", bufs=1) as tlp,
        tc.tile_pool(name="psg", bufs=2, space="PSUM") as psgp,
        tc.tile_pool(name="pst", bufs=2, space="PSUM") as pstp,
        tc.tile_pool(name="psm", bufs=1, space="PSUM") as psmp,
        tc.tile_pool(name="dram", bufs=1, space="DRAM") as dp,
    ):
        xt = cp.tile([96, 24 * 128], BF16, tag="xt")
        w2d0 = cp.tile([96, 24 * CV], BF16, tag="w2d0")
        wg = cp.tile([16, GTOT], BF16, tag="wg")
        xb = cp.tile([128, KW], BF16, tag="xb")
        w2 = cp.tile([96, 240 * 16], BF16, tag="w2")
        ident = cp.tile([128, 128], BF16, tag="ident")
        ident32 = cp.tile([128, 128], F32, tag="ident32")

        nc.sync.dma_start(ident[:], id_d[:])
        nc.sync.dma_start(xt[:, : 12 * 128], xt_d[:, : 12 * 128])
        nc.sync.dma_start(w2d0[:, : 12 * CV], w2d0_d[:, : 12 * CV])
        nc.sync.dma_start(xt[:, 12 * 128 :], xt_d[:, 12 * 128 :])
        nc.sync.dma_start(w2d0[:, 12 * CV :], w2d0_d[:, 12 * CV :])
        nc.sync.dma_start(ident32[:], id32_d[:])
        nc.sync.dma_start(wg[:], wg_d[:])
        nc.sync.dma_start(xb[:], xb_d[:])
        # w2 (ws weights) is deferred until after round 0 so the round-0
        # AllReduce DMAs don't queue behind it on SP
        if use_bias:
            biasr = cp.tile([128, IQ * C], BF16, tag="biasr")
            nc.scalar.dma_start(biasr[:], bias_d[:])

        # persistent small tiles
        warm = sp.tile([128, 1], F32, tag="warm")
        nc.vector.memset(warm[:], 1.0)
        warm2 = sp.tile([128, 1], F32, tag="warm2")
        epst = sp.tile([128, 1], F32, tag="epst")
        nc.vector.memset(epst[:], EPS)

        def prewarm(func, anchor=None):
            if anchor is None:
                nc.scalar.activation(warm2[:], warm[:], func)
            else:
                nc.scalar.activation(warm2[:], anchor, func, scale=0.0)

        prewarm(ACTF.Exp)

        la = sp.tile([128, IQ * C], BF16, tag="la")      # logits, (c,j) c-major
        et = sp.tile([128, IQ * C], BF16, tag="et")      # exp(logits)
        cw = sp.tile([128, IQ * C], BF16, tag="cw")      # couplings e (scaled)
        rz = sp.tile([128, IQ], BF16, tag="rz")
        v_b = sp.tile([128, CV], BF16, tag="v_b")        # routing vector v
        v_f = sp.tile([128, CV], F32, tag="v_f")         # final output
        s_part = sp.tile([128, CV], F32, tag="s_part")
        s_tot = sp.tile([128, CV], F32, tag="s_tot")
        sq = sp.tile([128, CV], F32, tag="sq")
        n2 = sp.tile([128, C], F32, tag="n2")
        lnb = sp.tile([128, C], F32, tag="lnb")
        rsq = sp.tile([128, C], F32, tag="rsq")
        m1 = sp.tile([128, C], F32, tag="m1")
        rm = sp.tile([128, C], F32, tag="rm")
        f0 = sp.tile([128, C], F32, tag="f0")
        fc = sp.tile([128, C], F32, tag="fc")
        sTe = [sp.tile([128, 128], F32, tag=f"sTe{g}", name=f"sTe{g}") for g in range(3)]
        vTk = sp.tile([16, C * 128], BF16, tag="vTk")    # vT slices, base-0 rows
        zp = [sp.tile([128, IQ], BF16, tag=f"zp{i}", name=f"zp{i}") for i in range(5)]
        zq = [sp.tile([128, IQ], BF16, tag=f"zq{i}", name=f"zq{i}") for i in range(2)]
        zf = sp.tile([128, IQ], BF16, tag="zf")
        zz = sp.tile([128, IQ], BF16, tag="zz")

        # psum bank-tiles, manually sub-allocated:
        #   m1t: sTA @0:128 | sTC @128:256
        #   m2t: sTB @0:128 | psrA @128:256 | psrB @256:384 | psrC+warm @384:512
        m1t = psmp.tile([128, 512], F32, tag="m1t")
        m2t = psmp.tile([128, 512], F32, tag="m2t")
        # the ws matmuls only write 16-row strips at 32-row spacing; the sTe
        # evictions read full tiles, so initialize the pad rows once
        nc.vector.memset(m1t[:], 0.0)
        nc.vector.memset(m2t[:], 0.0)
        ST_COL = {0: (m1t, 0), 1: (m2t, 0), 2: (m1t, 128)}
        PSR_COL = {0: (m2t, 128), 1: (m2t, 256), 2: (m2t, 384)}

        def all_reduce(tag, src_ap, dst_ap, n, q=None):
            q = q or nc.sync
            ccin = dp.tile([128, n], F32, tag=f"cci{tag}")
            ccout = dp.tile([128, n], F32, tag=f"cco{tag}")
            q.dma_start(ccin[:], src_ap)
            if cc_stub:
                q.dma_start(ccout[:], ccin[:])
            else:
                nc.gpsimd.collective_compute(
                    "AllReduce",
                    ALU.add,
                    replica_groups=REPLICA_GROUPS,
                    ins=[ccin.opt()],
                    outs=[ccout.opt()],
                )
            q.dma_start(dst_ap, ccout[:])

        def squash(lo, ncs, v_out):
            """squash s_tot[:, 16*lo : 16*(lo+ncs)] -> v_out (same cols).

            v = s * n2 / ((1+n2) * sqrt(n2+eps)). Sqrt switches the Act
            table set; boundaries are clustered so it loads once per round."""
            cs = slice(16 * lo, 16 * (lo + ncs))
            ss = slice(lo, lo + ncs)
            st = s_tot[:, cs]
            nc.scalar.activation(sq[:, cs], st, ACTF.Square)
            nc.vector.tensor_reduce(
                n2[:, ss], sq[:, cs].rearrange("p (c v) -> p c v", v=V),
                axis=AX.X, op=ALU.add,
            )
            nc.scalar.activation(rsq[:, ss], n2[:, ss], ACTF.Sqrt, bias=epst[:])
            nc.vector.scalar_tensor_tensor(
                m1[:, ss], n2[:, ss], 1.0, rsq[:, ss], op0=ALU.add, op1=ALU.mult
            )
            nc.vector.reciprocal(rm[:, ss], m1[:, ss])
            nc.vector.tensor_mul(fc[:, ss], n2[:, ss], rm[:, ss])
            f_b = _ap(fc[:, ss], [[1, ncs], [0, V]])
            nc.vector.tensor_mul(
                v_out[:, cs].rearrange("p (c v) -> p c v", v=V),
                st.rearrange("p (c v) -> p c v", v=V),
                f_b,
            )

        def vt_group(g, c0, ncs):
            """Transpose v_b[:, c] slices to base-0 rows in column slices of
            a cwt-ring psum tile; evict to vTk (bf16, partitions 0-15) so the
            g matmuls have lhsT and rhs both at partition base 0."""
            vtp = pstp.tile([128, 768], BF16, tag="cwt")
            for ci in range(ncs):
                c = c0 + ci
                nc.tensor.matmul(
                    vtp[0:16, ci * 128 : (ci + 1) * 128],
                    v_b[:, c * 16 : (c + 1) * 16],
                    ident[:],
                    start=True, stop=True,
                    is_transpose=True,
                    tile_position=(0, 0),
                )
            nc.vector.tensor_copy(
                vTk[:, c0 * 128 : (c0 + ncs) * 128], vtp[0:16, 0 : ncs * 128]
            )

        def pe_warm(n):
            # keep the PE p-state ramp hot across known idle joints (bf16
            # transposes of ident into the cwt psum ring)
            for _ in range(n):
                wtp = pstp.tile([128, 768], BF16, tag="cwt")
                nc.tensor.matmul(
                    wtp[:, 0:128],
                    ident[:],
                    ident[:],
                    start=True, stop=True,
                    is_transpose=True,
                    tile_position=(0, 0),
                )

        # ---- round 0: s0 = x . (W/C) folded dense matmul (c0 softmax folded
        # on host into w2d0); runs on xt tiles (d-major 96-row).
        pe_warm(10)
        ps0 = psgp.tile([128, CH], F32, tag="g")
        for t in range(24):
            nc.tensor.matmul(
                ps0[:, :CV],
                xt[:, t * 128 : (t + 1) * 128],
                w2d0[:, t * CV : (t + 1) * CV],
                start=(t == 0),
                stop=(t == 23),
            )
        nc.vector.tensor_copy(s_part[:], ps0[:, :CV])
        prewarm(ACTF.Sqrt, s_part[:, 0:1])   # S-set load hides under the AR
        # round-0 AllReduce split per c-group on three queues so the three
        # 3-hop DMA chains run concurrently; group A's squash/vT (and with
        # them round 1's A chunks) start while B/C are still in flight.
        for g, (c0, ncs) in enumerate(GROUPS):
            cs = slice(16 * c0, 16 * (c0 + ncs))
            q = (nc.sync, nc.gpsimd, nc.scalar)[g]
            all_reduce(f"r0g{g}", s_part[:, cs], s_tot[:, cs], 16 * ncs, q=q)
        for g, (c0, ncs) in enumerate(GROUPS):
            squash(c0, ncs, v_b)
            vt_group(g, c0, ncs)
        nc.sync.dma_start(w2[:], w2_d[:])
        prewarm(ACTF.Exp, v_b[:, 0:1])

        # chunk sub-spans: split [1024k, 1024k+len) at c (2304) and bank (512)
        # boundaries for the matmuls; at c boundaries only for the muls.
        def mm_subspans(k):
            lo = CH * k
            hi = min(lo + CH, GTOT)
            cuts = {lo, hi}
            for c in range(1, C):
                if lo < c * KW < hi:
                    cuts.add(c * KW)
            x = lo
            while x < hi:
                x = min((x // 512 + 1) * 512, hi)
                cuts.add(x)
            cuts = sorted(cuts)
            return [(cuts[i], cuts[i + 1]) for i in range(len(cuts) - 1)]

        def mul_subspans(k):
            lo = CH * k
            hi = min(lo + CH, GTOT)
            cuts = {lo, hi}
            for c in range(1, C):
                if lo < c * KW < hi:
                    cuts.add(c * KW)
            cuts = sorted(cuts)
            return [(cuts[i], cuts[i + 1]) for i in range(len(cuts) - 1)]

        # per-chunk g-eviction engine: Act by default; DVE -> the mul reads
        # the f32 psum directly (1x, but no evict); Pool -> gpsimd copy.
        DVE_DIRECT = {2, 7, 11, 16, 19, 22}
        POOL_EV = set()        # illegal: GPSIMD cannot access PSUM
        POOL_DIRECT = set()

        tmp_of = {}

        def emit_chunk(k, tmp, tb):
            """g matmuls for chunk k -> evict -> DVE mul into tmp."""
            gps = psgp.tile([128, CH], F32, tag="g")
            lo = CH * k
            w = min(CH, GTOT - lo)
            for (a, b2) in mm_subspans(k):
                c = a // KW
                nc.tensor.matmul(
                    gps[:, a - lo : b2 - lo],
                    vTk[:, c * 128 : (c + 1) * 128],
                    wg[:, a:b2],
                    start=True, stop=True,
                )
            if k in DVE_DIRECT or k in POOL_DIRECT:
                eng = nc.vector if k in DVE_DIRECT else nc.gpsimd
                for (a, b2) in mul_subspans(k):
                    c = a // KW
                    eng.tensor_mul(
                        tmp[:, a - tb : b2 - tb],
                        xb[:, a - c * KW : b2 - c * KW],
                        gps[:, a - lo : b2 - lo],
                    )
                return
            gb = gbp.tile([128, CH], BF16, tag="gb")
            if k in POOL_EV:
                nc.gpsimd.tensor_copy(gb[:, :w], gps[:, :w])
            else:
                nc.scalar.copy(gb[:, :w], gps[:, :w])
            for (a, b2) in mul_subspans(k):
                c = a // KW
                nc.vector.tensor_mul(
                    tmp[:, a - tb : b2 - tb],
                    xb[:, a - c * KW : b2 - c * KW],
                    gb[:, a - lo : b2 - lo],
                )

        def emit_group_chunks(rnd, gi):
            c0, ncs = GROUPS[gi]
            tag = "tmp" if ncs == 4 else "tmpc"
            tmp = tp.tile([128, ncs * KW], BF16, tag=tag, name=f"tmp{rnd}{gi}")
            tmp_of[(rnd, gi)] = tmp
            tb = c0 * KW
            for k in range(9 * gi, min(9 * (gi + 1), NCH)):
                emit_chunk(k, tmp, tb)

        def emit_tree(rnd, gi):
            """d-halving tree for group gi -> la slice (exp emitted apart)."""
            c0, ncs = GROUPS[gi]
            n = ncs * KW
            tmp = tmp_of[(rnd, gi)]
            tv = tmp[:, :n].rearrange("p (x d) -> p x d", d=8)
            t1 = tlp.tile([128, 4608], BF16, tag="t1")
            t1v = t1[:, : n // 2].rearrange("p (x d) -> p x d", d=4)
            nc.vector.tensor_add(t1v, tv[:, :, 0:4], tv[:, :, 4:8])
            t2 = tlp.tile([128, 2304], BF16, tag="t2")
            t2v = t2[:, : n // 4].rearrange("p (x d) -> p x d", d=2)
            nc.vector.tensor_add(t2v, t1v[:, :, 0:2], t1v[:, :, 2:4])
            ls = slice(c0 * IQ, (c0 + ncs) * IQ)
            lw = ncs * IQ
            ein = _ap(t2[:], [[2, lw]])
            ein2 = _ap(t2[:, 1:], [[2, lw]])
            if rnd == 1:
                nc.vector.tensor_add(la[:, ls], ein, ein2)
                if use_bias:
                    nc.gpsimd.tensor_add(la[:, ls], la[:, ls], biasr[:, ls])
            else:
                t3 = tlp.tile([128, 1152], BF16, tag="t3")
                nc.vector.tensor_add(t3[:, :lw], ein, ein2)
                nc.gpsimd.tensor_add(la[:, ls], la[:, ls], t3[:, :lw])

        def emit_exp(gi):
            c0, ncs = GROUPS[gi]
            ls = slice(c0 * IQ, (c0 + ncs) * IQ)
            nc.scalar.activation(et[:, ls], la[:, ls], ACTF.Exp)

        def emit_softmax_tail():
            """z-tree over c slices, rz, cw = et * rz (broadcast over c)."""
            ecs = [et[:, c * IQ : (c + 1) * IQ] for c in range(C)]
            for i in range(5):
                nc.vector.tensor_add(zp[i][:], ecs[2 * i], ecs[2 * i + 1])
            nc.vector.tensor_add(zq[0][:], zp[0][:], zp[1][:])
            nc.vector.tensor_add(zq[1][:], zp[2][:], zp[3][:])
            nc.vector.tensor_add(zf[:], zq[0][:], zq[1][:])
            nc.vector.tensor_add(zz[:], zf[:], zp[4][:])
            with nc.allow_low_precision(reason="1/z to bf16: couplings tolerate scale noise"):
                nc.vector.reciprocal(rz[:], zz[:])
            for g, (c0, ncs) in enumerate(GROUPS):
                ls = slice(c0 * IQ, (c0 + ncs) * IQ)
                rb = _ap(rz[:], [[0, ncs], [1, IQ]])
                nc.vector.tensor_mul(
                    cw[:, ls].rearrange("p (c j) -> p c j", c=ncs),
                    et[:, ls].rearrange("p (c j) -> p c j", c=ncs),
                    rb,
                )

        def emit_cwt_fill(f):
            """transpose cw tiles (c pair f) into bf16 psum [96, 6*128]."""
            cwt = pstp.tile([128, 768], BF16, tag="cwt")
            for t6 in range(6):
                tix = f * 6 + t6          # global (c, jt) tile index
                nc.tensor.matmul(
                    cwt[:96, t6 * 128 : (t6 + 1) * 128],
                    cw[:, tix * 96 : (tix + 1) * 96],
                    ident[:],
                    start=True, stop=True,
                    is_transpose=True,
                    tile_position=(0, 0),
                )
            return cwt

        POOL_WS = set()    # ws muls offloaded to the Pool engine (too slow)

        def emit_ws_c(c, cwt):
            """xc = cwT (.) xT (cw broadcast over d, read from bf16 psum);
            24 dense matmuls accumulate sT[(c,v), b]."""
            cof = (c % 2) * JT * 128
            xcq = xp.tile([96, 24 * 128], BF16, tag="xcq")
            in0 = _ap(cwt[:96, cof:], [[0, D], [128, JT], [1, 128]])
            in1 = _ap(xt[:], [[JT * 128, D], [128, JT], [1, 128]])
            out = _ap(xcq[:], [[JT * 128, D], [128, JT], [1, 128]])
            eng = nc.gpsimd if c in POOL_WS else nc.vector
            eng.tensor_mul(out, in0, in1)
            gi = 0 if c < 4 else (1 if c < 8 else 2)
            ci = c - GROUPS[gi][0]
            mt, col = ST_COL[gi]
            for tk in range(24):
                nc.tensor.matmul(
                    mt[32 * ci : 32 * ci + 16, col : col + 128],
                    w2[:, (c * 24 + tk) * 16 : (c * 24 + tk + 1) * 16],
                    xcq[:, tk * 128 : (tk + 1) * 128],
                    start=(tk == 0),
                    stop=(tk == 23),
                    tile_position=(0, 32 * ci),
                )

        def emit_boundary_pre(rnd, gi):
            """group gi: sT -> sTe -> transpose back -> s_part -> AR dmas.
            Emitted right after the group's last ws matmuls; the AR chain
            then runs under later ws/chunk work."""
            c0, ncs = GROUPS[gi]
            mt, col = ST_COL[gi]
            nc.scalar.copy(sTe[gi][:], mt[:, col : col + 128])
            pm, pcol = PSR_COL[gi]
            nc.tensor.matmul(
                pm[:, pcol : pcol + 128],
                sTe[gi][:],
                ident32[:],
                start=True, stop=True,
                is_transpose=True,
                tile_position=(0, 0),
            )
            cs = slice(16 * c0, 16 * (c0 + ncs))
            dstg = _ap(s_part[:, 16 * c0 :], [[16, ncs], [1, 16]])
            srcg = _ap(pm[:, pcol:], [[32, ncs], [1, 16]])
            nc.vector.tensor_copy(dstg, srcg)
            all_reduce(f"{rnd}g{gi}", s_part[:, cs], s_tot[:, cs], 16 * ncs)

        def emit_boundary_post(rnd, gi):
            """squash (waits on the AR); round 1 also vT for round 2.
            Deferred so its DVE/Act ops don't head-of-line block the queues
            while the AR is in flight."""
            c0, ncs = GROUPS[gi]
            if rnd == 1:
                squash(c0, ncs, v_b)
                vt_group(gi, c0, ncs)
            else:
                squash(c0, ncs, v_f)

        # ---- rounds 1 and 2, pipelined: round 2's g chunks for a c-group are
        # emitted right after round 1's boundary for that group, so they run
        # under round 1's remaining ws work.
        for gi in range(3):
            emit_group_chunks(1, gi)
            emit_tree(1, gi)
            emit_exp(gi)
        emit_softmax_tail()
        prewarm(ACTF.Sqrt, et[:, 0:1])
        cwt = emit_cwt_fill(0)
        emit_ws_c(0, cwt)
        emit_ws_c(1, cwt)
        cwt = emit_cwt_fill(1)
        emit_ws_c(2, cwt)
        emit_ws_c(3, cwt)
        emit_boundary_pre(1, 0)
        cwt = emit_cwt_fill(2)
        emit_ws_c(4, cwt)
        emit_ws_c(5, cwt)
        cwt = emit_cwt_fill(3)
        emit_ws_c(6, cwt)
        emit_ws_c(7, cwt)
        emit_boundary_pre(1, 1)
        emit_boundary_post(1, 0)      # AR-A done by now; vTk-A ready
        cwt = emit_cwt_fill(4)
        emit_ws_c(8, cwt)
        emit_group_chunks(2, 0)
        emit_ws_c(9, cwt)
        emit_boundary_pre(1, 2)
        emit_boundary_post(1, 1)
        emit_group_chunks(2, 1)
        emit_tree(2, 0)
        emit_boundary_post(1, 2)
        emit_group_chunks(2, 2)
        emit_tree(2, 1)
        emit_tree(2, 2)
        prewarm(ACTF.Exp, v_b[:, 159:160])
        for gi in range(3):
            emit_exp(gi)
        emit_softmax_tail()
        prewarm(ACTF.Sqrt, et[:, 0:1])
        cwt = emit_cwt_fill(0)
        emit_ws_c(0, cwt)
        emit_ws_c(1, cwt)
        cwt = emit_cwt_fill(1)
        emit_ws_c(2, cwt)
        emit_ws_c(3, cwt)
        emit_boundary_pre(2, 0)
        cwt = emit_cwt_fill(2)
        emit_ws_c(4, cwt)
        emit_ws_c(5, cwt)
        cwt = emit_cwt_fill(3)
        emit_ws_c(6, cwt)
        emit_ws_c(7, cwt)
        emit_boundary_pre(2, 1)
        emit_boundary_post(2, 0)
        cwt = emit_cwt_fill(4)
        emit_ws_c(8, cwt)
        emit_ws_c(9, cwt)
        emit_boundary_pre(2, 2)
        emit_boundary_post(2, 1)
        emit_boundary_post(2, 2)
        nc.sync.dma_start(out_d[:], v_f[:])


_PROGRAMS = {}


def _get_program(use_bias=False, cc_stub=False):
    key = (use_bias, cc_stub)
    if key not in _PROGRAMS:
        nc = bacc.Bacc(
            "TRN2", target_bir_lowering=False, debug=False, num_devices=8
        )
        with tile.TileContext(nc) as tc:
            _emit(nc, tc, use_bias, cc_stub)
        nc.compile()
        _PROGRAMS[key] = nc
    return _PROGRAMS[key]


def make_in_maps(inputs, W, bias):
    assert tuple(np.shape(inputs)) == (B, I, D), np.shape(inputs)
    assert tuple(np.shape(W)) == (I, C, D, V), np.shape(W)
    assert tuple(np.shape(bias)) == (1, I, C, 1), np.shape(bias)
    use_bias = bool(np.any(np.asarray(bias)))
    in_maps = []
    for k in range(8):
        bh, iq = k // 4, k % 4
        xs = np.asarray(inputs[bh * NB : (bh + 1) * NB, iq * IQ : (iq + 1) * IQ, :])
        ws = np.asarray(W[iq * IQ : (iq + 1) * IQ])  # [288, 10, 8, 16] (j,c,d,v)

        # xb: b-form x, (j, d) j-major
        xb = xs.reshape(NB, IQ * D)

        # xt: T-form x, (d, jt) 96-row tiles -> [96, 24*128]
        A = xs.transpose(2, 1, 0).reshape(D * IQ, NB)   # row = d*288 + j
        xt = A.reshape(24, 96, NB).transpose(1, 0, 2).reshape(96, 24 * 128)

        # wg: [16 v, (c, j, d)] for the g matmuls
        wgm = ws.transpose(3, 1, 0, 2).reshape(V, C * IQ * D)

        # w2: ws weights, dense [96, 16] tiles indexed (c*24 + d*3 + jt)
        Wc = ws.transpose(1, 2, 0, 3)  # [c, d, j, v]
        w2t = Wc.reshape(C, D, JT, 96, V).transpose(3, 0, 1, 2, 4)
        w2 = w2t.reshape(96, C * D * JT * V)

        # w2d0: round-0 dense weights on xt tiles (d, jt), with the round-0
        # softmax couplings folded in (uniform 1/C for zero bias)
        bs = np.asarray(bias[0, iq * IQ : (iq + 1) * IQ, :, 0], dtype=np.float64)
        eb = np.exp(bs - bs.max(axis=1, keepdims=True))
        cb = (eb / eb.sum(axis=1, keepdims=True)).astype(np.float32)  # [288, 10]
        Wt_s = ws.transpose(0, 2, 1, 3) * cb[:, None, :, None]  # [j, d, c, v]
        w2d0 = (
            Wt_s.transpose(1, 0, 2, 3)
            .reshape(D, JT, 96, CV)
            .transpose(2, 0, 1, 3)
            .reshape(96, 24 * CV)
        )

        ident = np.eye(128, dtype=np.float32)

        m = {
            "xt": np.ascontiguousarray(xt).astype(ml_dtypes.bfloat16),
            "w2d0": np.ascontiguousarray(w2d0).astype(ml_dtypes.bfloat16),
            "wg": np.ascontiguousarray(wgm).astype(ml_dtypes.bfloat16),
            "xb": np.ascontiguousarray(xb).astype(ml_dtypes.bfloat16),
            "w2": np.ascontiguousarray(w2).astype(ml_dtypes.bfloat16),
            "ident": ident.astype(ml_dtypes.bfloat16),
            "ident32": ident,
        }
        if use_bias:
            bsr = np.asarray(bias[0, iq * IQ : (iq + 1) * IQ, :, 0])  # [288, 10]
            biasr = np.broadcast_to(
                np.ascontiguousarray(bsr.T).reshape(1, IQ * C), (128, IQ * C)
            )
            m["biasr"] = np.ascontiguousarray(biasr).astype(ml_dtypes.bfloat16)
        in_maps.append(m)
    return use_bias, in_maps


def run(inputs, W, bias, **kw):
    use_bias, in_maps = make_in_maps(inputs, W, bias)
    nc = _get_program(use_bias)
    res = run_bass_kernel_spmd(nc, in_maps, core_ids=list(range(8)), **kw)
    outs = res.results
    o0 = np.asarray(outs[0]["out"], dtype=np.float32).reshape(NB, C, V)
    o1 = np.asarray(outs[4]["out"], dtype=np.float32).reshape(NB, C, V)
    return np.concatenate([o0, o1], axis=0), res


def kernel(inputs, W, bias):
    out, _ = run(inputs, W, bias)
    return out
